# revision 8
# baseline (speedup 1.0000x reference)
"""DiagonalLSTM Trainium2 kernel — band-restricted scan, fp16 split matmuls.

Sharding: data-parallel over batch B=16 across 8 cores (2 batch elems/core).
Per-core layout: partitions = 128-wide HID gate chunks, free dim = (b, j)
where j indexes the LIVE DIAGONAL BAND rows [lo..hi], lo = max(0, t-63),
hi = min(t, 63).  Rows r > t share the zero-input state v_t (host fp64
table) seeded into row t+1 each step; rows r < lo are dead but their last
written value (step r+63) is exactly what the shifted tap needs next step.

Matmul precision: fp16 runs 1 cycle/row on the PE vs fp32's 4 (and fp32r is
also 4 below 256-wide), but raw fp16 rounding (2^-11) is chaotically
amplified by the 127-step scan.  Every scan matmul is a 3-term Dekker split
with power-of-2 scale 64 keeping all lo-parts out of fp16 subnormal range
(unscaled splits lose ~8 bits to subnormal quantization):

    w @ h  ~=  w_hi @ h_hi  +  (w_lo*64) @ (h_hi/64)  +  (w_hi/64) @ (h_lo*64)

Host pre-splits weights and x; the device splits h each step.  Products are
exact in the PE, PSUM accumulates fp32, so scan accuracy stays near fp32.
The x-side correction pair is K-stacked into ONE K=128 matmul:
[wis_lo*64; wis_hi/64] @ [x_hi/64; x_lo*64].  The gate bias rides two
ones-rows of the K=66 main x matmul (hi + lo*64 parts, ones row values
1 and 1/64).

State tiles use a GUARD-COLUMN layout [128, 2*(H+1)]: each batch block is
[guard, r0..r63] with the guard kept zero, so the shifted (r-1) taps and
the c_sh elementwise term read the guard instead of needing edge-restricted
access patterns.  This keeps every PSUM matmul output contiguous [[1, n]]
— partial-coverage strided PSUM writes hit accumulation hazards in the
zero-region hardware (and the simulator's checker).

fp32r is NOT used anywhere: an fp32r matmul operand triggers an in-place
RNE-12 rounding pass that corrupts adjacent SBUF bytes (fp16 tiles here).
The feed-forward residual matmul runs in plain fp16 instead (error enters
once, ~2^-11, fine for the output).

Sigmoid gates as 0.5*(1+tanh(z/2)) via pre-halved weights; ONE tanh per
chunk pair; emission order (i, g) -> (fl, fu) -> (o) so the c-chain's
inputs close earliest.  State convention: h2 = 2h, c2 = 2c; res accumulates
2*(h + residual) and the host halves the output.
"""

import numpy as np

import concourse.bass as bass
import concourse.mybir as mybir
from concourse import bacc
from concourse import tile
from concourse.bass_utils import run_bass_kernel_spmd

B, C, H, W = 16, 64, 64, 64
HID = 128
SW = H + W - 1  # 127
NCORES = 8
BL = B // NCORES  # 2
H1 = H + 1         # 65: guard col + 64 rows per block
NST = BL * H1      # 130 state cols
NRES = BL * H * W  # 8192 output cols

F32 = mybir.dt.float32
F16 = mybir.dt.float16
AF = mybir.ActivationFunctionType
ALU = mybir.AluOpType

S = 64.0
RS = 1.0 / 64.0

# band geometry per step (shared host/device)
_LO = [max(0, t - (W - 1)) for t in range(SW)]
_HI = [min(t, H - 1) for t in range(SW)]
_M = [hi - lo + 1 for lo, hi in zip(_LO, _HI)]
_BASE = np.concatenate([[0], np.cumsum([BL * m for m in _M])]).astype(int)
XC = int(_BASE[-1])  # 8192

# emission order: i, g (c-chain critical) then fl, fu then o.
# chunk gate indices: 0=o, 1=fl, 2=fu, 3=i, 4=g.  th block = emission idx.
EORD = (3, 4, 1, 2, 0)
TH_I, TH_G, TH_FL, TH_FU, TH_O = 0, 1, 2, 3, 4  # th blocks by emission slot


def _raw(t, off, dims):
    """Raw AP on tile t: keep its partition pair, custom free dims."""
    return bass.AP(t.tensor, t.offset + off, [list(t.ap[0])] + [list(d) for d in dims])


def build_program():
    nc = bacc.Bacc(None, target_bir_lowering=False)

    xmain_d = nc.dram_tensor("xmain", [C + 2, XC], F32, kind="ExternalInput")
    xres_d = nc.dram_tensor("xres", [C + 1, NRES], F16, kind="ExternalInput")
    wtap_hi_d = nc.dram_tensor("wtap_hi", [HID, 2 * 5 * HID], F16, kind="ExternalInput")
    wtap_lo_d = nc.dram_tensor("wtap_lo", [HID, 2 * 5 * HID], F16, kind="ExternalInput")
    wtap_hs_d = nc.dram_tensor("wtap_hs", [HID, 2 * 5 * HID], F16, kind="ExternalInput")
    wis_m_d = nc.dram_tensor("wis_m", [C + 2, 5 * HID], F32, kind="ExternalInput")
    wres_d = nc.dram_tensor("wres", [C + 1, HID], F16, kind="ExternalInput")
    # virgin-state seed tables, columns duplicated for the 2-col strided copy
    hvh_d = nc.dram_tensor("hvh", [HID, 2 * (H - 1)], F16, kind="ExternalInput")
    hvs_d = nc.dram_tensor("hvs", [HID, 2 * (H - 1)], F16, kind="ExternalInput")
    hvl_d = nc.dram_tensor("hvl", [HID, 2 * (H - 1)], F16, kind="ExternalInput")
    cv_d = nc.dram_tensor("cv", [HID, 2 * (H - 1)], F32, kind="ExternalInput")
    out_d = nc.dram_tensor("out", [HID, NRES], F32, kind="ExternalOutput")

    with tile.TileContext(nc) as tc:
        with (
            tc.tile_pool(name="const", bufs=1) as const,
            tc.tile_pool(name="state", bufs=3) as state,
            tc.tile_pool(name="tmp", bufs=3) as tmp,
            tc.tile_pool(name="gpsumA", bufs=3, space="PSUM") as gpsumA,
            tc.tile_pool(name="gpsumO", bufs=2, space="PSUM") as gpsumO,
        ):
            xmain = const.tile([C + 2, XC], F32)
            xres = const.tile([C + 1, NRES], F16)
            wtap_hi = const.tile([HID, 2 * 5 * HID], F16)
            wtap_lo = const.tile([HID, 2 * 5 * HID], F16)
            wtap_hs = const.tile([HID, 2 * 5 * HID], F16)
            wis_m = const.tile([C + 2, 5 * HID], F32)
            wres = const.tile([C + 1, HID], F16)
            hvh = const.tile([HID, 2 * (H - 1)], F16)
            hvs = const.tile([HID, 2 * (H - 1)], F16)
            hvl = const.tile([HID, 2 * (H - 1)], F16)
            cv = const.tile([HID, 2 * (H - 1)], F32)
            res = const.tile([HID, NRES], F32)

            nc.sync.dma_start(out=wis_m, in_=wis_m_d[:])
            nc.sync.dma_start(out=wtap_hi, in_=wtap_hi_d[:])
            nc.sync.dma_start(out=wtap_lo, in_=wtap_lo_d[:])
            nc.sync.dma_start(out=wtap_hs, in_=wtap_hs_d[:])
            nc.sync.dma_start(out=hvh, in_=hvh_d[:])
            nc.sync.dma_start(out=hvs, in_=hvs_d[:])
            nc.sync.dma_start(out=hvl, in_=hvl_d[:])
            nc.sync.dma_start(out=cv, in_=cv_d[:])
            nc.sync.dma_start(out=wres, in_=wres_d[:])
            steps_cut = [0, 4, 12, 24, 36, 48, 64, 80, 100, SW]
            for a, b in zip(steps_cut[:-1], steps_cut[1:]):
                lo_e, hi_e = int(_BASE[a]), int(_BASE[b])
                nc.sync.dma_start(out=xmain[:, lo_e:hi_e], in_=xmain_d[:, lo_e:hi_e])
            nc.sync.dma_start(out=xres, in_=xres_d[:])

            def pbankA():
                ps = gpsumA.tile([HID, 1024], F32, tag="A")
                return ps

            def pbankO():
                ps = gpsumO.tile([HID, 512], F32, tag="O")
                return ps

            # ---- scan state: guard layout [HID, BL*(H+1)] ----
            c_cur = state.tile([HID, NST], F32, tag="c")
            hh_cur = state.tile([HID, NST], F16, tag="hh")  # h2 hi (fp16)
            hs_cur = state.tile([HID, NST], F16, tag="hs")  # h2 hi / 64
            hl_cur = state.tile([HID, NST], F16, tag="hl")  # (h2 - hi) * 64
            nc.vector.memzero(c_cur)
            nc.vector.memzero(hh_cur)
            nc.vector.memzero(hs_cur)
            nc.vector.memzero(hl_cur)

            def SV(ap, a, b):
                """State view: block-index range [a..b) (guard at 0, row r at
                r+1 per block)."""
                return ap.rearrange("p (b r) -> p b r", b=BL)[:, :, a:b]

            def slots(tiles):
                a1, a2, po = tiles
                return ((a1, 0), (a1, 512), (a2, 0), (a2, 512), (po, 0))

            def xmm(t):
                """i_s matmuls for step t: packed band cols (opens groups).
                K=66 fp32 exact (x + bias ones-row)."""
                b0, n = int(_BASE[t]), BL * _M[t]
                tiles = (pbankA(), pbankA(), pbankO())
                for idx, k in enumerate(EORD):
                    pk, off = slots(tiles)[idx]
                    nc.tensor.matmul(
                        _raw(pk, off, [[1, n]]),
                        wis_m[:, k * HID:(k + 1) * HID],
                        xmain[:, b0:b0 + n],
                        start=True, stop=False,
                    )
                return tiles

            pcur = xmm(0)

            for t in range(SW):
                lo, hi, m = _LO[t], _HI[t], _M[t]
                n = BL * m

                th = tmp.tile([HID, 5 * HID], F32, tag="th")
                for idx, k in enumerate(EORD):
                    pk, off = slots(pcur)[idx]
                    # taps: (wtap, h-source) pairs of the 3-term split;
                    # w1 reads rows [lo..hi] (idx lo+1..), w0 the shifted
                    # rows [lo-1..hi-1] (idx lo..; guard/dead rows supply
                    # the edge values).  All outputs contiguous [[1, n]].
                    for j, (wt, hsrc) in enumerate((
                        (wtap_hi, hh_cur),
                        (wtap_lo, hs_cur),
                        (wtap_hs, hl_cur),
                    )):
                        w0c = wt[:, k * HID:(k + 1) * HID]
                        w1c = wt[:, 5 * HID + k * HID:5 * HID + (k + 1) * HID]
                        nc.tensor.matmul(
                            _raw(pk, off, [[1, n]]),
                            w0c,
                            SV(hsrc, lo, hi + 1),
                            start=False, stop=False,
                        )
                        nc.tensor.matmul(
                            _raw(pk, off, [[1, n]]),
                            w1c,
                            SV(hsrc, lo + 1, hi + 2),
                            start=False, stop=(j == 2),
                        )
                    # paired tanh fires when both banks of the pair close
                    if idx in (1, 3):
                        nc.scalar.activation(
                            _raw(th, (idx - 1) * HID, [[HID, 2], [1, n]]),
                            _raw(pk, 0, [[512, 2], [1, n]]),
                            AF.Tanh,
                        )
                    elif idx == 4:
                        nc.scalar.activation(
                            _raw(th, 4 * HID, [[1, n]]),
                            _raw(pk, 0, [[1, n]]),
                            AF.Tanh,
                        )

                # prefetch next step's x-side matmuls while ACT/DVE run
                if t + 1 < SW:
                    pcur = xmm(t + 1)

                # DVE c-chain (th blocks: 0=i, 1=g, 2=fl, 3=fu, 4=o):
                # r = (t_i+1)*g ; P = (t_fl+1)*C2 + (t_fu+1)*C2sh ;
                # C2' = 0.5*P + r  on band rows
                r_t = tmp.tile([HID, HID], F32, tag="r_t")
                nc.vector.scalar_tensor_tensor(
                    _raw(r_t, 0, [[1, n]]),
                    _raw(th, TH_I * HID, [[1, n]]),
                    1.0, _raw(th, TH_G * HID, [[1, n]]), op0=ALU.add, op1=ALU.mult,
                )
                p = tmp.tile([HID, HID], F32, tag="p")
                nc.vector.scalar_tensor_tensor(
                    _raw(p, 0, [[m, BL], [1, m]]),
                    _raw(th, TH_FL * HID, [[m, BL], [1, m]]),
                    1.0, SV(c_cur, lo + 1, hi + 2), op0=ALU.add, op1=ALU.mult,
                )
                q = tmp.tile([HID, HID], F32, tag="q")
                nc.vector.scalar_tensor_tensor(
                    _raw(q, 0, [[m, BL], [1, m]]),
                    _raw(th, TH_FU * HID, [[m, BL], [1, m]]),
                    1.0, SV(c_cur, lo, hi + 1), op0=ALU.add, op1=ALU.mult,
                )
                nc.vector.tensor_add(
                    _raw(p, 0, [[1, n]]), _raw(p, 0, [[1, n]]), _raw(q, 0, [[1, n]]),
                )
                c_new = state.tile([HID, NST], F32, tag="c")
                if t < 2:
                    nc.vector.memzero(c_new)
                nc.vector.scalar_tensor_tensor(
                    SV(c_new, lo + 1, hi + 2),
                    _raw(p, 0, [[m, BL], [1, m]]),
                    0.5, _raw(r_t, 0, [[m, BL], [1, m]]),
                    op0=ALU.mult, op1=ALU.add,
                )

                tanc = tmp.tile([HID, HID], F32, tag="tanc")
                nc.scalar.activation(
                    _raw(tanc, 0, [[1, n]]),
                    SV(c_new, lo + 1, hi + 2), AF.Tanh, scale=0.5,
                )
                # h split: hh = fp16((th_o+1)*tanc) [chain], hs = hh/64,
                # h2 fp32 and its lo-part on Pool (off-chain)
                hh_new = state.tile([HID, NST], F16, tag="hh")
                hs_new = state.tile([HID, NST], F16, tag="hs")
                hl_new = state.tile([HID, NST], F16, tag="hl")
                if t < 2:
                    nc.vector.memzero(hh_new)
                    nc.vector.memzero(hs_new)
                    nc.vector.memzero(hl_new)
                nc.vector.scalar_tensor_tensor(
                    SV(hh_new, lo + 1, hi + 2),
                    _raw(th, TH_O * HID, [[m, BL], [1, m]]),
                    1.0, _raw(tanc, 0, [[m, BL], [1, m]]),
                    op0=ALU.add, op1=ALU.mult,
                )
                dummy = tmp.tile([HID, 1], F32, tag="dummy")
                nc.vector.affine_mul_reduce(
                    SV(hs_new, lo + 1, hi + 2),
                    dummy,
                    _raw(th, TH_O * HID, [[m, BL], [1, m]]),
                    _raw(tanc, 0, [[m, BL], [1, m]]),
                    RS, RS,
                )
                # h2 fp32 (res diagonal source + lo-part source), then
                # hl = (h2 - hh)*64; res write on Pool
                h2 = tmp.tile([HID, HID], F32, tag="h2")
                nc.vector.scalar_tensor_tensor(
                    _raw(h2, 0, [[1, n]]),
                    _raw(th, TH_O * HID, [[1, n]]),
                    1.0, _raw(tanc, 0, [[1, n]]), op0=ALU.add, op1=ALU.mult,
                )
                res_ap = _raw(res, (W - 1) * lo + t, [[H * W, BL], [W - 1, m]])
                nc.gpsimd.tensor_copy(out=res_ap, in_=_raw(h2, 0, [[m, BL], [1, m]]))
                d32 = tmp.tile([HID, HID], F32, tag="d32")
                nc.vector.tensor_sub(
                    _raw(d32, 0, [[m, BL], [1, m]]),
                    _raw(h2, 0, [[m, BL], [1, m]]),
                    SV(hh_new, lo + 1, hi + 2),
                )
                nc.vector.tensor_scalar_mul(
                    SV(hl_new, lo + 1, hi + 2),
                    _raw(d32, 0, [[m, BL], [1, m]]),
                    S,
                )

                # seed the virgin row t+1 (block idx t+2) for the next step
                if t + 1 <= H - 1:
                    for tbl, dst in ((hvh, hh_new), (hvs, hs_new),
                                     (hvl, hl_new), (cv, c_new)):
                        nc.gpsimd.tensor_copy(
                            out=_raw(dst, t + 2, [[H1, BL], [1, 1]]),
                            in_=tbl[:, 2 * t:2 * t + 2],
                        )

                c_cur = c_new
                hh_cur, hs_cur, hl_cur = hh_new, hs_new, hl_new

                # Late-scan interleave: once an 8-row block's diagonal cells
                # are all written (t = 8j+70), add its residual (fp16 matmul;
                # feed-forward so single rounding is safe) and DMA it out.
                if t >= 70 and (t - 70) % 8 == 0 and (t - 70) // 8 < 8:
                    j = (t - 70) // 8
                    for b in range(BL):
                        cols = slice(b * H * W + 512 * j, b * H * W + 512 * j + 512)
                        rp = pbankO()
                        nc.tensor.matmul(
                            rp, wres, xres[:, cols], start=True, stop=True
                        )
                        nc.vector.tensor_add(res[:, cols], res[:, cols], rp)
                        nc.sync.dma_start(out=out_d[:, cols], in_=res[:, cols])

    nc.finalize()
    return nc


_NC_CACHE = {}


def _get_nc():
    if "nc" not in _NC_CACHE:
        _NC_CACHE["nc"] = build_program()
    return _NC_CACHE["nc"]


def _split16(a, scale_lo=S):
    """hi = fp16(a); lo = fp16((a - hi) * scale_lo). Exact Dekker split."""
    a = np.asarray(a, np.float32)
    hi = a.astype(np.float16)
    lo = ((a - hi.astype(np.float32)) * np.float32(scale_lo)).astype(np.float16)
    return hi, lo


def _virgin_tables(w_ss, b_is, b_ss):
    """fp64 recurrence for the shared zero-input state v_t, t = 0..62.

    Returns device-convention tables (2h, 2c), [HID, 63] fp64."""
    w0 = np.asarray(w_ss, np.float64)[:, :, 0]
    w1 = np.asarray(w_ss, np.float64)[:, :, 1]
    bb = np.asarray(b_is, np.float64) + np.asarray(b_ss, np.float64)
    wsum = w0 + w1
    h = np.zeros(HID)
    c = np.zeros(HID)
    hv = np.zeros((HID, H - 1), np.float64)
    cvt = np.zeros((HID, H - 1), np.float64)
    for t in range(H - 1):
        z = bb + wsum @ h
        o, fl, fu, i, g = np.split(z, 5)
        sig = lambda v: 1.0 / (1.0 + np.exp(-v))
        o, fl, fu, i = sig(o), sig(fl), sig(fu), sig(i)
        c = fl * c + fu * c + i * np.tanh(g)
        h = o * np.tanh(c)
        hv[:, t] = 2.0 * h
        cvt[:, t] = 2.0 * c
    return hv, cvt


def _prep_inputs(x, w_is, b_is, w_ss, b_ss, w_res, b_res):
    x = np.asarray(x, np.float32)
    # band-packed skewed x: col _BASE[t] + b*m + (r - lo) = x[b, :, r, t - r]
    xs = x.reshape(NCORES, BL, C, H, W)
    xsk = np.zeros((NCORES, C, XC), np.float32)
    for t in range(SW):
        lo, hi, m = _LO[t], _HI[t], _M[t]
        rows = np.arange(lo, hi + 1)
        blk = xs[:, :, :, rows, t - rows]          # [cores, BL, C, m]
        blk = blk.transpose(0, 2, 1, 3)            # [cores, C, BL, m]
        xsk[:, :, _BASE[t]:_BASE[t + 1]] = blk.reshape(NCORES, C, BL * m)
    ones = np.ones((NCORES, 1, XC), np.float32)
    xmain = np.concatenate(
        [xsk, ones, np.zeros((NCORES, 1, XC), np.float32)], axis=1)

    xres = np.asarray(x).reshape(NCORES, BL, C, H, W).transpose(0, 2, 1, 3, 4)
    xres = xres.reshape(NCORES, C, NRES)
    xres = np.concatenate([xres, np.ones((NCORES, 1, NRES), np.float32)], axis=1)
    xres = xres.astype(np.float16)

    # gate scaling: chunks 0..3 (o, f_left, f_up, i) are sigmoid gates,
    # computed via tanh(z/2) -> pre-halve their weights and biases.
    gs = np.ones((5 * HID,), np.float32)
    gs[0:4 * HID] = 0.5

    # wtap[i, tap*640 + o] = w_ss[o, i, tap] * gs[o] * 0.5
    # (extra 0.5: the kernel's h state holds 2h)
    wtap = np.asarray(w_ss, np.float32).transpose(1, 2, 0) * (0.5 * gs)[None, None, :]
    wtap = np.ascontiguousarray(wtap.reshape(HID, 2 * 5 * HID), np.float32)
    wtap_hi, wtap_lo = _split16(wtap)              # lo pre-scaled by 64
    wtap_hs = (wtap_hi.astype(np.float32) * RS).astype(np.float16)

    bvec = ((np.asarray(b_is, np.float32) + np.asarray(b_ss, np.float32)) * gs)
    wis = np.asarray(w_is, np.float32).T * gs[None, :]
    wis_m = np.ascontiguousarray(np.concatenate(
        [wis, bvec[None, :], np.zeros((1, 5 * HID), np.float32)], axis=0))

    # x2: the device residual tile accumulates 2*(residual + sum h); the
    # host halves the final output.
    wres = 2.0 * np.concatenate(
        [np.asarray(w_res, np.float32).T, np.asarray(b_res, np.float32)[None, :]],
        axis=0,
    ).astype(np.float32)
    wres = wres.astype(np.float16)

    hv, cvt = _virgin_tables(w_ss, b_is, b_ss)
    hvh16 = hv.astype(np.float32).astype(np.float16)
    hvl16 = ((hv.astype(np.float32) - hvh16.astype(np.float32)) * S).astype(np.float16)
    hvs16 = (hvh16.astype(np.float32) * RS).astype(np.float16)
    dup = lambda a, dt: np.ascontiguousarray(
        np.repeat(np.asarray(a, np.float32), 2, axis=1)).astype(dt)
    hvh_t = dup(hvh16, np.float16)
    hvs_t = dup(hvs16, np.float16)
    hvl_t = dup(hvl16, np.float16)
    cv_t = dup(cvt, np.float32)

    in_maps = []
    for cix in range(NCORES):
        in_maps.append({
            "xmain": np.ascontiguousarray(xmain[cix]),
            "xres": np.ascontiguousarray(xres[cix]),
            "wtap_hi": wtap_hi, "wtap_lo": wtap_lo, "wtap_hs": wtap_hs,
            "wis_m": wis_m,
            "wres": wres,
            "hvh": hvh_t, "hvs": hvs_t, "hvl": hvl_t, "cv": cv_t,
        })
    return in_maps


def kernel(x, w_is, b_is, w_ss, b_ss, w_res, b_res, _trace=False):
    nc = _get_nc()
    in_maps = _prep_inputs(x, w_is, b_is, w_ss, b_ss, w_res, b_res)
    r = run_bass_kernel_spmd(nc, in_maps, list(range(NCORES)), trace=_trace)
    outs = [r.results[c]["out"] for c in range(NCORES)]
    out = np.stack(outs, 0).reshape(NCORES, HID, BL, H, W)
    out = out.transpose(0, 2, 1, 3, 4).reshape(B, HID, H, W)
    return np.ascontiguousarray(out * np.float32(0.5))


# revision 10
# speedup vs baseline: 1.0404x; 1.0404x over previous
"""DiagonalLSTM Trainium2 kernel — band-restricted scan, fp16 split matmuls.

Sharding: data-parallel over batch B=16 across 8 cores (2 batch elems/core).
Per-core layout: partitions = 128-wide HID gate chunks, free dim = (b, j)
where j indexes the LIVE DIAGONAL BAND rows [lo..hi], lo = max(0, t-63),
hi = min(t, 63).  Rows r > t share the zero-input state v_t (host fp64
table) seeded into row t+1 each step; rows r < lo are dead but their last
written value (step r+63) is exactly what the shifted tap needs next step.

Matmul precision: fp16 runs 1 cycle/row on the PE vs fp32's 4 (and fp32r is
also 4 below 256-wide), but raw fp16 rounding (2^-11) is chaotically
amplified by the 127-step scan.  Every scan matmul is a 3-term Dekker split
with power-of-2 scale 64 keeping all lo-parts out of fp16 subnormal range
(unscaled splits lose ~8 bits to subnormal quantization):

    w @ h  ~=  w_hi @ h_hi  +  (w_lo*64) @ (h_hi/64)  +  (w_hi/64) @ (h_lo*64)

Host pre-splits weights and x; the device splits h each step.  Products are
exact in the PE, PSUM accumulates fp32, so scan accuracy stays near fp32.
The x-side correction pair is K-stacked into ONE K=128 matmul:
[wis_lo*64; wis_hi/64] @ [x_hi/64; x_lo*64].  The gate bias rides two
ones-rows of the K=66 main x matmul (hi + lo*64 parts, ones row values
1 and 1/64).

State tiles use a GUARD-COLUMN layout [128, 2*(H+1)]: each batch block is
[guard, r0..r63] with the guard kept zero, so the shifted (r-1) taps and
the c_sh elementwise term read the guard instead of needing edge-restricted
access patterns.  This keeps every PSUM matmul output contiguous [[1, n]]
— partial-coverage strided PSUM writes hit accumulation hazards in the
zero-region hardware (and the simulator's checker).

fp32r is NOT used anywhere: an fp32r matmul operand triggers an in-place
RNE-12 rounding pass that corrupts adjacent SBUF bytes (fp16 tiles here).
The feed-forward residual matmul runs in plain fp16 instead (error enters
once, ~2^-11, fine for the output).

Sigmoid gates as 0.5*(1+tanh(z/2)) via pre-halved weights; ONE tanh per
chunk pair; emission order (i, g) -> (fl, fu) -> (o) so the c-chain's
inputs close earliest.  State convention: h2 = 2h, c2 = 2c; res accumulates
2*(h + residual) and the host halves the output.
"""

import numpy as np

import concourse.bass as bass
import concourse.mybir as mybir
from concourse import bacc
from concourse import tile
from concourse.bass_utils import run_bass_kernel_spmd

B, C, H, W = 16, 64, 64, 64
HID = 128
SW = H + W - 1  # 127
NCORES = 8
BL = B // NCORES  # 2
H1 = H + 1         # 65: guard col + 64 rows per block
NST = BL * H1      # 130 state cols
NRES = BL * H * W  # 8192 output cols

F32 = mybir.dt.float32
F16 = mybir.dt.float16
AF = mybir.ActivationFunctionType
ALU = mybir.AluOpType

S = 64.0
RS = 1.0 / 64.0

# band geometry per step (shared host/device)
_LO = [max(0, t - (W - 1)) for t in range(SW)]
_HI = [min(t, H - 1) for t in range(SW)]
_M = [hi - lo + 1 for lo, hi in zip(_LO, _HI)]
_BASE = np.concatenate([[0], np.cumsum([BL * m for m in _M])]).astype(int)
XC = int(_BASE[-1])  # 8192

# emission order: i, g (c-chain critical) then fl, fu then o.
# chunk gate indices: 0=o, 1=fl, 2=fu, 3=i, 4=g.  th block = emission idx.
EORD = (3, 4, 1, 2, 0)
TH_I, TH_G, TH_FL, TH_FU, TH_O = 0, 1, 2, 3, 4  # th blocks by emission slot


def _raw(t, off, dims):
    """Raw AP on tile t: keep its partition pair, custom free dims."""
    return bass.AP(t.tensor, t.offset + off, [list(t.ap[0])] + [list(d) for d in dims])


def build_program():
    nc = bacc.Bacc(None, target_bir_lowering=False)

    xmain_d = nc.dram_tensor("xmain", [C + 2, XC], F32, kind="ExternalInput")
    xres_d = nc.dram_tensor("xres", [C + 1, NRES], F16, kind="ExternalInput")
    wtap_hi_d = nc.dram_tensor("wtap_hi", [HID, 2 * 5 * HID], F16, kind="ExternalInput")
    wtap_lo_d = nc.dram_tensor("wtap_lo", [HID, 2 * 5 * HID], F16, kind="ExternalInput")
    wtap_hs_d = nc.dram_tensor("wtap_hs", [HID, 2 * 5 * HID], F16, kind="ExternalInput")
    wis_m_d = nc.dram_tensor("wis_m", [C + 2, 5 * HID], F32, kind="ExternalInput")
    wres_d = nc.dram_tensor("wres", [C + 1, HID], F16, kind="ExternalInput")
    # virgin-state seed tables, columns duplicated for the 2-col strided copy
    hvh_d = nc.dram_tensor("hvh", [HID, 2 * (H - 1)], F16, kind="ExternalInput")
    hvs_d = nc.dram_tensor("hvs", [HID, 2 * (H - 1)], F16, kind="ExternalInput")
    hvl_d = nc.dram_tensor("hvl", [HID, 2 * (H - 1)], F16, kind="ExternalInput")
    cv_d = nc.dram_tensor("cv", [HID, 2 * (H - 1)], F32, kind="ExternalInput")
    kc_d = nc.dram_tensor("kc", [HID, 2], F32, kind="ExternalInput")
    out_d = nc.dram_tensor("out", [HID, NRES], F32, kind="ExternalOutput")

    with tile.TileContext(nc) as tc:
        with (
            tc.tile_pool(name="const", bufs=1) as const,
            tc.tile_pool(name="state", bufs=3) as state,
            tc.tile_pool(name="tmp", bufs=3) as tmp,
            tc.tile_pool(name="gpsumA", bufs=3, space="PSUM") as gpsumA,
            tc.tile_pool(name="gpsumO", bufs=2, space="PSUM") as gpsumO,
        ):
            xmain = const.tile([C + 2, XC], F32)
            xres = const.tile([C + 1, NRES], F16)
            wtap_hi = const.tile([HID, 2 * 5 * HID], F16)
            wtap_lo = const.tile([HID, 2 * 5 * HID], F16)
            wtap_hs = const.tile([HID, 2 * 5 * HID], F16)
            wis_m = const.tile([C + 2, 5 * HID], F32)
            wres = const.tile([C + 1, HID], F16)
            hvh = const.tile([HID, 2 * (H - 1)], F16)
            hvs = const.tile([HID, 2 * (H - 1)], F16)
            hvl = const.tile([HID, 2 * (H - 1)], F16)
            cv = const.tile([HID, 2 * (H - 1)], F32)
            kc = const.tile([HID, 2], F32)
            res = const.tile([HID, NRES], F32)

            nc.sync.dma_start(out=wis_m, in_=wis_m_d[:])
            nc.sync.dma_start(out=wtap_hi, in_=wtap_hi_d[:])
            nc.sync.dma_start(out=wtap_lo, in_=wtap_lo_d[:])
            nc.sync.dma_start(out=wtap_hs, in_=wtap_hs_d[:])
            nc.sync.dma_start(out=hvh, in_=hvh_d[:])
            nc.sync.dma_start(out=hvs, in_=hvs_d[:])
            nc.sync.dma_start(out=hvl, in_=hvl_d[:])
            nc.sync.dma_start(out=cv, in_=cv_d[:])
            nc.sync.dma_start(out=kc, in_=kc_d[:])
            nc.sync.dma_start(out=wres, in_=wres_d[:])
            steps_cut = [0, 4, 12, 24, 36, 48, 64, 80, 100, SW]
            for a, b in zip(steps_cut[:-1], steps_cut[1:]):
                lo_e, hi_e = int(_BASE[a]), int(_BASE[b])
                nc.sync.dma_start(out=xmain[:, lo_e:hi_e], in_=xmain_d[:, lo_e:hi_e])
            nc.sync.dma_start(out=xres, in_=xres_d[:])

            def pbankA():
                ps = gpsumA.tile([HID, 1024], F32, tag="A")
                return ps

            def pbankO():
                ps = gpsumO.tile([HID, 512], F32, tag="O")
                return ps

            # ---- scan state: guard layout [HID, BL*(H+1)] ----
            c_cur = state.tile([HID, NST], F32, tag="c")
            hh_cur = state.tile([HID, NST], F16, tag="hh")  # h2 hi (fp16)
            hs_cur = state.tile([HID, NST], F16, tag="hs")  # h2 hi / 64
            hl_cur = state.tile([HID, NST], F16, tag="hl")  # (h2 - hi) * 64
            nc.vector.memzero(c_cur)
            nc.vector.memzero(hh_cur)
            nc.vector.memzero(hs_cur)
            nc.vector.memzero(hl_cur)

            def SV(ap, a, b):
                """State view: block-index range [a..b) (guard at 0, row r at
                r+1 per block)."""
                return ap.rearrange("p (b r) -> p b r", b=BL)[:, :, a:b]

            def slots(tiles):
                a1, a2, po = tiles
                return ((a1, 0), (a1, 512), (a2, 0), (a2, 512), (po, 0))

            def xmm(t):
                """i_s matmuls for step t: packed band cols (opens groups).
                K=66 fp32 exact (x + bias ones-row)."""
                b0, n = int(_BASE[t]), BL * _M[t]
                tiles = (pbankA(), pbankA(), pbankO())
                for idx, k in enumerate(EORD):
                    pk, off = slots(tiles)[idx]
                    nc.tensor.matmul(
                        _raw(pk, off, [[1, n]]),
                        wis_m[:, k * HID:(k + 1) * HID],
                        xmain[:, b0:b0 + n],
                        start=True, stop=False,
                    )
                return tiles

            pcur = xmm(0)

            for t in range(SW):
                lo, hi, m = _LO[t], _HI[t], _M[t]
                n = BL * m

                th = tmp.tile([HID, 5 * HID], F32, tag="th")
                for idx, k in enumerate(EORD):
                    pk, off = slots(pcur)[idx]
                    # taps: (wtap, h-source) pairs of the 3-term split;
                    # w1 reads rows [lo..hi] (idx lo+1..), w0 the shifted
                    # rows [lo-1..hi-1] (idx lo..; guard/dead rows supply
                    # the edge values).  All outputs contiguous [[1, n]].
                    for j, (wt, hsrc) in enumerate((
                        (wtap_hi, hh_cur),
                        (wtap_lo, hs_cur),
                        (wtap_hs, hl_cur),
                    )):
                        w0c = wt[:, k * HID:(k + 1) * HID]
                        w1c = wt[:, 5 * HID + k * HID:5 * HID + (k + 1) * HID]
                        nc.tensor.matmul(
                            _raw(pk, off, [[1, n]]),
                            w0c,
                            SV(hsrc, lo, hi + 1),
                            start=False, stop=False,
                        )
                        nc.tensor.matmul(
                            _raw(pk, off, [[1, n]]),
                            w1c,
                            SV(hsrc, lo + 1, hi + 2),
                            start=False, stop=(j == 2),
                        )
                    # paired tanh fires when both banks of the pair close
                    if idx in (1, 3):
                        nc.scalar.activation(
                            _raw(th, (idx - 1) * HID, [[HID, 2], [1, n]]),
                            _raw(pk, 0, [[512, 2], [1, n]]),
                            AF.Tanh,
                        )
                    elif idx == 4:
                        nc.scalar.activation(
                            _raw(th, 4 * HID, [[1, n]]),
                            _raw(pk, 0, [[1, n]]),
                            AF.Tanh,
                        )

                # prefetch next step's x-side matmuls while ACT/DVE run
                if t + 1 < SW:
                    pcur = xmm(t + 1)

                # DVE c-chain (th blocks: 0=i, 1=g, 2=fl, 3=fu, 4=o):
                # r = (t_i+1)*g ; P = (t_fl+1)*C2 + (t_fu+1)*C2sh ;
                # C2' = 0.5*P + r  on band rows
                r_t = tmp.tile([HID, HID], F32, tag="r_t")
                nc.vector.scalar_tensor_tensor(
                    _raw(r_t, 0, [[1, n]]),
                    _raw(th, TH_I * HID, [[1, n]]),
                    1.0, _raw(th, TH_G * HID, [[1, n]]), op0=ALU.add, op1=ALU.mult,
                )
                p = tmp.tile([HID, HID], F32, tag="p")
                nc.vector.scalar_tensor_tensor(
                    _raw(p, 0, [[m, BL], [1, m]]),
                    _raw(th, TH_FL * HID, [[m, BL], [1, m]]),
                    1.0, SV(c_cur, lo + 1, hi + 2), op0=ALU.add, op1=ALU.mult,
                )
                q = tmp.tile([HID, HID], F32, tag="q")
                nc.vector.scalar_tensor_tensor(
                    _raw(q, 0, [[m, BL], [1, m]]),
                    _raw(th, TH_FU * HID, [[m, BL], [1, m]]),
                    1.0, SV(c_cur, lo, hi + 1), op0=ALU.add, op1=ALU.mult,
                )
                nc.vector.tensor_add(
                    _raw(p, 0, [[1, n]]), _raw(p, 0, [[1, n]]), _raw(q, 0, [[1, n]]),
                )
                c_new = state.tile([HID, NST], F32, tag="c")
                if t < 2:
                    nc.vector.memzero(c_new)
                nc.vector.scalar_tensor_tensor(
                    SV(c_new, lo + 1, hi + 2),
                    _raw(p, 0, [[m, BL], [1, m]]),
                    0.5, _raw(r_t, 0, [[m, BL], [1, m]]),
                    op0=ALU.mult, op1=ALU.add,
                )

                tanc = tmp.tile([HID, HID], F32, tag="tanc")
                nc.scalar.activation(
                    _raw(tanc, 0, [[1, n]]),
                    SV(c_new, lo + 1, hi + 2), AF.Tanh, scale=0.5,
                )
                # h split: hh = fp16((th_o+1)*tanc) [chain], hs = hh/64,
                # h2 fp32 and its lo-part on Pool (off-chain)
                hh_new = state.tile([HID, NST], F16, tag="hh")
                hs_new = state.tile([HID, NST], F16, tag="hs")
                hl_new = state.tile([HID, NST], F16, tag="hl")
                if t < 2:
                    nc.vector.memzero(hh_new)
                    nc.vector.memzero(hs_new)
                    nc.vector.memzero(hl_new)
                nc.vector.scalar_tensor_tensor(
                    SV(hh_new, lo + 1, hi + 2),
                    _raw(th, TH_O * HID, [[m, BL], [1, m]]),
                    1.0, _raw(tanc, 0, [[m, BL], [1, m]]),
                    op0=ALU.add, op1=ALU.mult,
                )
                nc.gpsimd.tensor_mul(
                    SV(hs_new, lo + 1, hi + 2),
                    SV(hh_new, lo + 1, hi + 2),
                    bass.AP(kc.tensor, kc.offset,
                            [list(kc.ap[0]), [0, BL], [0, m]]),
                )
                # h2 fp32 (res diagonal source + lo-part source), then
                # hl = (h2 - hh)*64; res write on Pool
                h2 = tmp.tile([HID, HID], F32, tag="h2")
                nc.vector.scalar_tensor_tensor(
                    _raw(h2, 0, [[1, n]]),
                    _raw(th, TH_O * HID, [[1, n]]),
                    1.0, _raw(tanc, 0, [[1, n]]), op0=ALU.add, op1=ALU.mult,
                )
                res_ap = _raw(res, (W - 1) * lo + t, [[H * W, BL], [W - 1, m]])
                nc.gpsimd.tensor_copy(out=res_ap, in_=_raw(h2, 0, [[m, BL], [1, m]]))
                d32 = tmp.tile([HID, HID], F32, tag="d32")
                nc.gpsimd.tensor_sub(
                    _raw(d32, 0, [[m, BL], [1, m]]),
                    _raw(h2, 0, [[m, BL], [1, m]]),
                    SV(hh_new, lo + 1, hi + 2),
                )
                nc.gpsimd.tensor_mul(
                    SV(hl_new, lo + 1, hi + 2),
                    _raw(d32, 0, [[m, BL], [1, m]]),
                    bass.AP(kc.tensor, kc.offset + 1,
                            [list(kc.ap[0]), [0, BL], [0, m]]),
                )

                # seed the virgin row t+1 (block idx t+2) for the next step
                if t + 1 <= H - 1:
                    for tbl, dst in ((hvh, hh_new), (hvs, hs_new),
                                     (hvl, hl_new), (cv, c_new)):
                        nc.gpsimd.tensor_copy(
                            out=_raw(dst, t + 2, [[H1, BL], [1, 1]]),
                            in_=tbl[:, 2 * t:2 * t + 2],
                        )

                c_cur = c_new
                hh_cur, hs_cur, hl_cur = hh_new, hs_new, hl_new

                # Late-scan interleave: once an 8-row block's diagonal cells
                # are all written (t = 8j+70), add its residual (fp16 matmul;
                # feed-forward so single rounding is safe) and DMA it out.
                if t >= 70 and (t - 70) % 8 == 0 and (t - 70) // 8 < 8:
                    j = (t - 70) // 8
                    for b in range(BL):
                        cols = slice(b * H * W + 512 * j, b * H * W + 512 * j + 512)
                        rp = pbankO()
                        nc.tensor.matmul(
                            rp, wres, xres[:, cols], start=True, stop=True
                        )
                        nc.vector.tensor_add(res[:, cols], res[:, cols], rp)
                        nc.sync.dma_start(out=out_d[:, cols], in_=res[:, cols])

    nc.finalize()
    return nc


_NC_CACHE = {}


def _get_nc():
    if "nc" not in _NC_CACHE:
        _NC_CACHE["nc"] = build_program()
    return _NC_CACHE["nc"]


def _split16(a, scale_lo=S):
    """hi = fp16(a); lo = fp16((a - hi) * scale_lo). Exact Dekker split."""
    a = np.asarray(a, np.float32)
    hi = a.astype(np.float16)
    lo = ((a - hi.astype(np.float32)) * np.float32(scale_lo)).astype(np.float16)
    return hi, lo


def _virgin_tables(w_ss, b_is, b_ss):
    """fp64 recurrence for the shared zero-input state v_t, t = 0..62.

    Returns device-convention tables (2h, 2c), [HID, 63] fp64."""
    w0 = np.asarray(w_ss, np.float64)[:, :, 0]
    w1 = np.asarray(w_ss, np.float64)[:, :, 1]
    bb = np.asarray(b_is, np.float64) + np.asarray(b_ss, np.float64)
    wsum = w0 + w1
    h = np.zeros(HID)
    c = np.zeros(HID)
    hv = np.zeros((HID, H - 1), np.float64)
    cvt = np.zeros((HID, H - 1), np.float64)
    for t in range(H - 1):
        z = bb + wsum @ h
        o, fl, fu, i, g = np.split(z, 5)
        sig = lambda v: 1.0 / (1.0 + np.exp(-v))
        o, fl, fu, i = sig(o), sig(fl), sig(fu), sig(i)
        c = fl * c + fu * c + i * np.tanh(g)
        h = o * np.tanh(c)
        hv[:, t] = 2.0 * h
        cvt[:, t] = 2.0 * c
    return hv, cvt


def _prep_inputs(x, w_is, b_is, w_ss, b_ss, w_res, b_res):
    x = np.asarray(x, np.float32)
    # band-packed skewed x: col _BASE[t] + b*m + (r - lo) = x[b, :, r, t - r]
    xs = x.reshape(NCORES, BL, C, H, W)
    xsk = np.zeros((NCORES, C, XC), np.float32)
    for t in range(SW):
        lo, hi, m = _LO[t], _HI[t], _M[t]
        rows = np.arange(lo, hi + 1)
        blk = xs[:, :, :, rows, t - rows]          # [cores, BL, C, m]
        blk = blk.transpose(0, 2, 1, 3)            # [cores, C, BL, m]
        xsk[:, :, _BASE[t]:_BASE[t + 1]] = blk.reshape(NCORES, C, BL * m)
    ones = np.ones((NCORES, 1, XC), np.float32)
    xmain = np.concatenate(
        [xsk, ones, np.zeros((NCORES, 1, XC), np.float32)], axis=1)

    xres = np.asarray(x).reshape(NCORES, BL, C, H, W).transpose(0, 2, 1, 3, 4)
    xres = xres.reshape(NCORES, C, NRES)
    xres = np.concatenate([xres, np.ones((NCORES, 1, NRES), np.float32)], axis=1)
    xres = xres.astype(np.float16)

    # gate scaling: chunks 0..3 (o, f_left, f_up, i) are sigmoid gates,
    # computed via tanh(z/2) -> pre-halve their weights and biases.
    gs = np.ones((5 * HID,), np.float32)
    gs[0:4 * HID] = 0.5

    # wtap[i, tap*640 + o] = w_ss[o, i, tap] * gs[o] * 0.5
    # (extra 0.5: the kernel's h state holds 2h)
    wtap = np.asarray(w_ss, np.float32).transpose(1, 2, 0) * (0.5 * gs)[None, None, :]
    wtap = np.ascontiguousarray(wtap.reshape(HID, 2 * 5 * HID), np.float32)
    wtap_hi, wtap_lo = _split16(wtap)              # lo pre-scaled by 64
    wtap_hs = (wtap_hi.astype(np.float32) * RS).astype(np.float16)

    bvec = ((np.asarray(b_is, np.float32) + np.asarray(b_ss, np.float32)) * gs)
    wis = np.asarray(w_is, np.float32).T * gs[None, :]
    wis_m = np.ascontiguousarray(np.concatenate(
        [wis, bvec[None, :], np.zeros((1, 5 * HID), np.float32)], axis=0))

    # x2: the device residual tile accumulates 2*(residual + sum h); the
    # host halves the final output.
    wres = 2.0 * np.concatenate(
        [np.asarray(w_res, np.float32).T, np.asarray(b_res, np.float32)[None, :]],
        axis=0,
    ).astype(np.float32)
    wres = wres.astype(np.float16)

    hv, cvt = _virgin_tables(w_ss, b_is, b_ss)
    hvh16 = hv.astype(np.float32).astype(np.float16)
    hvl16 = ((hv.astype(np.float32) - hvh16.astype(np.float32)) * S).astype(np.float16)
    hvs16 = (hvh16.astype(np.float32) * RS).astype(np.float16)
    dup = lambda a, dt: np.ascontiguousarray(
        np.repeat(np.asarray(a, np.float32), 2, axis=1)).astype(dt)
    hvh_t = dup(hvh16, np.float16)
    hvs_t = dup(hvs16, np.float16)
    hvl_t = dup(hvl16, np.float16)
    cv_t = dup(cvt, np.float32)

    kc = np.tile(np.array([[RS, S]], np.float32), (HID, 1))
    in_maps = []
    for cix in range(NCORES):
        in_maps.append({
            "xmain": np.ascontiguousarray(xmain[cix]),
            "xres": np.ascontiguousarray(xres[cix]),
            "wtap_hi": wtap_hi, "wtap_lo": wtap_lo, "wtap_hs": wtap_hs,
            "wis_m": wis_m,
            "wres": wres,
            "hvh": hvh_t, "hvs": hvs_t, "hvl": hvl_t, "cv": cv_t, "kc": kc,
        })
    return in_maps


def kernel(x, w_is, b_is, w_ss, b_ss, w_res, b_res, _trace=False):
    nc = _get_nc()
    in_maps = _prep_inputs(x, w_is, b_is, w_ss, b_ss, w_res, b_res)
    r = run_bass_kernel_spmd(nc, in_maps, list(range(NCORES)), trace=_trace)
    outs = [r.results[c]["out"] for c in range(NCORES)]
    out = np.stack(outs, 0).reshape(NCORES, HID, BL, H, W)
    out = out.transpose(0, 2, 1, 3, 4).reshape(B, HID, H, W)
    return np.ascontiguousarray(out * np.float32(0.5))
